# revision 43
# baseline (speedup 1.0000x reference)
"""CTC loss kernel for Trainium2, 8-core batch-parallel.

Per core (2 batch elements, no cross-core comms):
  1. Stream logits [2,800,5000]; ScalarE exp+accum -> per-t sumexp; Ln +
     length-masked sum via PE ones-matmul  => sum_{t<T_b} lse[b,t].
  2. Gather the 100 target-label columns per batch with dynamic-offset DMAs
     (label id read from SBUF into an engine register at runtime -> SPMD-safe),
     exp, multiply by host weight (per-t rescale schedule x valid-label mask),
     skew-copy so chunk j's labels sit at slot l+j.
  3. Wavefront DP: alpha(t,s) = (alpha(t-1,s)+c(t-1,s))*p(t,s) as one
     tensor_tensor_scan over t per (state s, chunk j); 16 chunks x 2 batches
     on 32 partitions; waves w = s + 2j; chunk boundary carried by a
     stream_shuffle partition shift.
  4. Captures at t = T_b-1 via dynamic-offset DMAs; loss assembled on device.

Linear-domain DP with a per-t rescale schedule C_b(t) (offline fit of the
length-parametrized drift shape) keeps alpha in fp32 range; the loss adds
back sum C_b(t) and subtracts sum lse.
"""

import numpy as np

B, T, V, L = 16, 800, 5000, 100
TC = 50
NCH = 16
NLANE = 32
NS = 201
WAVES = NS + 2 * (NCH - 1)   # 231
NSLOT = WAVES + 2
LAYW = TC + 1
ASTORE_W = NSLOT * LAYW
NSKEW = L + NCH              # 116

UB = [0.,0.,0.2,0.4,0.6,0.8,1.,1.2,1.4,1.6,1.8,2.,2.2,2.4,2.6,2.8,3.,3.2,3.4,
      3.6,3.8,4.,4.2,4.4,4.6,4.8,5.,5.2,5.4,5.6,5.8,6.,6.2,6.4,6.6,6.8,7.,7.2,
      7.4,7.6,7.8,8.,8.2,8.4,8.6,8.8,9.,9.2,9.4,9.6,9.8,10.,10.2,10.4,10.6,
      10.8,11.,11.2,11.4,11.6,11.8,12.,12.2,12.4,12.6,12.8,13.,13.2,13.4,13.6,
      13.8,14.,14.2,14.4,14.6,14.8,15.,15.2,15.4,15.6,15.8,16.]
FV = [1.3931,1.3931,1.3931,1.3931,1.3744,1.375,1.3635,1.3448,1.3604,1.3452,
      1.3477,1.3034,1.2414,1.1511,1.0982,1.0283,0.9665,0.8953,0.8484,0.7696,
      0.8067,0.8122,0.7825,0.7641,0.7564,0.7072,0.6458,0.6282,0.6271,0.6359,
      0.6205,0.6547,0.6475,0.645,0.6361,0.6556,0.5972,0.5117,0.4888,0.4881,
      0.5372,0.5809,0.6505,0.6021,0.5623,0.567,0.5426,0.5163,0.4476,0.4424,
      0.4354,0.3862,0.3193,0.3114,0.3286,0.336,0.3488,0.4185,0.527,0.471,
      0.5321,0.5321,0.5321,0.5321,0.5321,0.5321,0.5321,0.5321,0.5321,0.5321,
      0.5321,0.5321,0.5321,0.5321,0.5321,0.5321,0.5321,0.5321,0.5321,0.5321,
      0.5321,0.5321]
COEF = [0.030914305653999778, 0.3095809144838023,
        -0.00012226065581205555, -59.652794100809864]


def _sched(tl_b, il_b):
    u = np.arange(T, dtype=np.float64) / float(tl_b)
    base = np.interp(u, np.asarray(UB), np.asarray(FV))
    x = np.array([1.0, tl_b / il_b, float(tl_b), 1.0 / il_b])
    return base + float(x @ np.asarray(COEF))


_PROGRAM = None


def _build_program():
    global _PROGRAM
    if _PROGRAM is not None:
        return _PROGRAM
    from contextlib import ExitStack
    import concourse.bacc as bacc
    import concourse.bass as bass
    import concourse.tile as tile
    import concourse.mybir as mybir

    f32 = mybir.dt.float32
    i32 = mybir.dt.int32
    AL = mybir.AluOpType
    AF = mybir.ActivationFunctionType

    nc = bacc.Bacc(detect_race_conditions=False)

    logits_d = nc.dram_tensor("logits", [2, T, V], f32, kind="ExternalInput")
    targ_d = nc.dram_tensor("targ", [1, 2 * L], i32, kind="ExternalInput")
    wfac_d = nc.dram_tensor("wfac", [NLANE, L * TC], f32, kind="ExternalInput")
    wfacpb_d = nc.dram_tensor("wfacpb", [NLANE, TC], f32, kind="ExternalInput")
    skipg_d = nc.dram_tensor("skipg", [NLANE, NSKEW], f32, kind="ExternalInput")
    masklse_d = nc.dram_tensor("masklse", [128, 14], f32, kind="ExternalInput")
    constc_d = nc.dram_tensor("constc", [1, 2], f32, kind="ExternalInput")
    capidx_d = nc.dram_tensor("capidx", [1, 8], i32, kind="ExternalInput")
    lanemask_d = nc.dram_tensor("lanemask", [NLANE, 4], f32,
                                kind="ExternalInput")
    # col 0: DP seed (1.0 at lanes 0,16); col 1: boundary zero-mask
    seedz_d = nc.dram_tensor("seedz", [NLANE, 2], f32, kind="ExternalInput")
    loss_d = nc.dram_tensor("loss", [1, 2], f32, kind="ExternalOutput")

    es = ExitStack()
    # raw SBUF tensors that persist across TileContexts (full 128 partitions
    # so their byte ranges never alias under rectangle packing)
    p_raw = es.enter_context(nc.sbuf_tensor([128, L * TC], f32))
    pb_raw = es.enter_context(nc.sbuf_tensor([128, TC], f32))
    targ_sb = es.enter_context(nc.sbuf_tensor([128, 2 * L], i32))

    gsem = nc.alloc_semaphore("gsem")
    gsem_sw = nc.alloc_semaphore("gsem_sw")  # SWDGE sems must start at 0
    gcnt = 0
    gcnt_sw = 0

    def gather_ctx(lrange):
        nonlocal gcnt, gcnt_sw
        with tile.TileContext(nc) as tc:
            engs = [nc.sync, nc.scalar, nc.gpsimd]
            for gi, (bl, l) in enumerate(lrange):
                eng = engs[gi % 3]
                sw = eng is nc.gpsimd
                src3 = logits_d[bl].rearrange("(j c) v -> j c v", c=TC)
                with tc.tile_critical(), nc.allow_non_contiguous_dma(
                        reason="label-column gather is inherently strided"):
                    val = eng.value_load(
                        targ_sb[0:1, bl * L + l:bl * L + l + 1])
                    eng.dma_start(
                        p_raw[16 * bl:16 * bl + NCH, l * TC:(l + 1) * TC],
                        src3[:, :, bass.ds(val, 1)].rearrange(
                            "j c one -> j (c one)"),
                    ).then_inc(gsem_sw if sw else gsem, 16)
                if sw:
                    gcnt_sw += 16
                else:
                    gcnt += 16

    # ---- phase 0: targ load + blank columns ----
    with tile.TileContext(nc) as tc:
        with tc.tile_critical():
            nc.sync.dma_start(
                targ_sb[0:1, :], targ_d[:]).then_inc(gsem, 16)
        gcnt += 16
        for bl in range(2):
            src3 = logits_d[bl].rearrange("(j c) v -> j c v", c=TC)
            with tc.tile_critical(), nc.allow_non_contiguous_dma(
                    reason="blank-column gather is inherently strided"):
                nc.sync.dma_start(
                    pb_raw[16 * bl:16 * bl + NCH, 0:TC],
                    src3[:, :, 0:1].rearrange("j c one -> j (c one)"),
                ).then_inc(gsem, 16)
            gcnt += 16
        # all gather engines wait for targ + blank columns before reading
        with tc.tile_critical():
            nc.sync.wait_ge(gsem, gcnt)
            nc.scalar.wait_ge(gsem, gcnt)
            nc.gpsimd.wait_ge(gsem, gcnt)

    # ---- gather phases (register pools flush at each context exit) ----
    pairs = [(bl, l) for bl in range(2) for l in range(L)]
    gather_ctx(pairs[:100])
    gather_ctx(pairs[100:])
    GTOTAL = gcnt
    GTOTAL_SW = gcnt_sw

    # ---- main phase ----
    with tile.TileContext(nc) as tc:
        with (
            tc.tile_pool(name="big", bufs=2) as bigp,
            tc.tile_pool(name="pers", bufs=1) as pp,
            tc.tile_pool(name="psum", bufs=1, space="PSUM") as psp,
            tc.tile_pool(name="dram", bufs=1, space="DRAM") as drp,
        ):
            astore = pp.tile([NLANE, ASTORE_W], f32, tag="astore")
            p_exp = pp.tile([NLANE, L * TC], f32, tag="p_exp")
            p_skew = pp.tile([NLANE, NSKEW * TC], f32, tag="p_skew")
            pb_exp = pp.tile([NLANE, TC], f32, tag="pb_exp")
            wfac_sb = pp.tile([NLANE, L * TC], f32, tag="wfac_sb")
            wfacpb_sb = pp.tile([NLANE, TC], f32, tag="wfacpb_sb")
            skipg_sb = pp.tile([NLANE, NSKEW], f32, tag="skipg_sb")
            lse_sb = pp.tile([128, 14], f32, tag="lse_sb")
            masklse_sb = pp.tile([128, 14], f32, tag="masklse_sb")
            ones_sb = pp.tile([128, 1], f32, tag="ones_sb")
            scratch = pp.tile([128, V], f32, tag="scratch")
            bshift = pp.tile([NLANE, 1], f32, tag="bshift")
            inj = pp.tile([NLANE, TC], f32, tag="inj")
            tmp1 = pp.tile([NLANE, TC], f32, tag="tmp1")
            capt = pp.tile([NLANE, 4], f32, tag="capt")
            lanemask_sb = pp.tile([NLANE, 4], f32, tag="lanemask_sb")
            seedz_sb = pp.tile([NLANE, 2], f32, tag="seedz_sb")
            psum_cap = psp.tile([1, 4], f32, tag="psum_cap")
            capidx_sb = pp.tile([1, 8], i32, tag="capidx_sb")
            constc_sb = pp.tile([1, 2], f32, tag="constc_sb")
            smalls = pp.tile([1, 16], f32, tag="smalls")
            lsered = pp.tile([1, 14], f32, tag="lsered")
            psum_lse = psp.tile([1, 14], f32, tag="psum_lse")

            nc.sync.dma_start(wfac_sb[:], wfac_d[:])
            nc.sync.dma_start(wfacpb_sb[:], wfacpb_d[:])
            nc.sync.dma_start(skipg_sb[:], skipg_d[:])
            nc.sync.dma_start(masklse_sb[:], masklse_d[:])
            nc.sync.dma_start(capidx_sb[:], capidx_d[:])
            nc.sync.dma_start(constc_sb[:], constc_d[:])
            nc.sync.dma_start(lanemask_sb[:], lanemask_d[:])
            nc.sync.dma_start(seedz_sb[:], seedz_d[:])

            nc.vector.memset(astore[:], 0.0)
            nc.vector.memset(p_skew[:], 0.0)
            nc.vector.memset(lse_sb[:], 1.0)
            nc.vector.memset(ones_sb[:], 1.0)
            nc.vector.tensor_copy(
                astore[:, 2 * LAYW:2 * LAYW + 1], seedz_sb[:, 0:1])

            # exp of gathered label/blank logits (after all gathers landed)
            with tc.tile_critical():
                nc.scalar.wait_ge(gsem, GTOTAL)
                nc.scalar.wait_ge(gsem_sw, GTOTAL_SW)
                nc.scalar.activation(p_exp[:], p_raw[0:NLANE, :], AF.Exp)
                nc.scalar.activation(pb_exp[:], pb_raw[0:NLANE, :], AF.Exp)
            nc.vector.tensor_tensor(
                out=p_exp[:], in0=p_exp[:], in1=wfac_sb[:], op=AL.mult)
            nc.vector.tensor_tensor(
                out=pb_exp[:], in0=pb_exp[:], in1=wfacpb_sb[:], op=AL.mult)

            for bl in range(2):
                for j in range(NCH):
                    ln = 16 * bl + j
                    nc.sync.dma_start(
                        p_skew[ln:ln + 1, j * TC:(j + L) * TC],
                        p_exp[ln:ln + 1, 0:L * TC],
                    )

            # ---- streaming lse ----
            for bl in range(2):
                for k in range(7):
                    nk = min(128, T - 128 * k)
                    lt = bigp.tile([128, V], f32, tag="lt")
                    nc.sync.dma_start(
                        lt[0:nk, :], logits_d[bl, 128 * k:128 * k + nk, :])
                    nc.scalar.activation(
                        scratch[0:nk, :], lt[0:nk, :], AF.Exp,
                        accum_out=lse_sb[0:nk, bl * 7 + k:bl * 7 + k + 1])

            nc.scalar.activation(lse_sb[:], lse_sb[:], AF.Ln)
            nc.vector.tensor_tensor(
                out=lse_sb[:], in0=lse_sb[:], in1=masklse_sb[:], op=AL.mult)
            nc.tensor.matmul(
                psum_lse[:], lhsT=ones_sb[:], rhs=lse_sb[:],
                start=True, stop=True)
            nc.scalar.copy(lsered[:], psum_lse[:])
            nc.vector.tensor_reduce(
                smalls[0:1, 14:15], lsered[0:1, 0:7],
                axis=mybir.AxisListType.X, op=AL.add)
            nc.vector.tensor_reduce(
                smalls[0:1, 15:16], lsered[0:1, 7:14],
                axis=mybir.AxisListType.X, op=AL.add)

            # ---- wavefront DP ----
            # shift by one partition within each 16-lane half
            SHUF = [0] + list(range(15)) + [16] + list(range(16, 31))
            for w in range(WAVES):
                slot_w = w + 2
                c_out = slot_w * LAYW
                c_prev = (slot_w - 1) * LAYW
                c_pp = (slot_w - 2) * LAYW
                if w % 2 == 0:
                    nc.vector.tensor_tensor_scan(
                        out=astore[:, c_out + 1:c_out + 1 + TC],
                        data0=astore[:, c_prev:c_prev + TC],
                        data1=pb_exp[:, 0:TC],
                        initial=astore[:, c_out:c_out + 1],
                        op0=AL.add, op1=AL.mult)
                else:
                    sl = (w - 1) // 2
                    nc.vector.tensor_scalar(
                        out=tmp1[:, 0:TC],
                        in0=astore[:, c_pp:c_pp + TC],
                        scalar1=skipg_sb[:, sl:sl + 1],
                        scalar2=None, op0=AL.mult)
                    nc.vector.tensor_tensor(
                        out=inj[:, 0:TC], in0=tmp1[:, 0:TC],
                        in1=astore[:, c_prev:c_prev + TC], op=AL.add)
                    nc.vector.tensor_tensor_scan(
                        out=astore[:, c_out + 1:c_out + 1 + TC],
                        data0=inj[:, 0:TC],
                        data1=p_skew[:, sl * TC:(sl + 1) * TC],
                        initial=astore[:, c_out:c_out + 1],
                        op0=AL.add, op1=AL.mult)
                if w + 2 < WAVES:
                    c_nxt = (slot_w + 2) * LAYW
                    nc.vector.stream_shuffle(
                        out=bshift[:, 0:1],
                        in_=astore[:, c_out + TC:c_out + TC + 1],
                        mask=SHUF)
                    nc.vector.tensor_tensor(
                        out=bshift[:, 0:1], in0=bshift[:, 0:1],
                        in1=seedz_sb[:, 1:2], op=AL.mult)
                    nc.vector.tensor_copy(
                        astore[:, c_nxt:c_nxt + 1], bshift[:, 0:1])

            # ---- captures: full 32-lane column at dynamic col ----
            # SBUF-source dynamic DMAs fail on HW; bounce alpha through DRAM
            # and capture with (supported) DRAM-source dynamic DMAs.
            astore_dram = drp.tile([NLANE, ASTORE_W], f32, tag="astore_dram")
            csem = nc.alloc_semaphore("csem")
            with tc.tile_critical():
                nc.sync.dma_start(
                    astore_dram[:], astore[:]).then_inc(csem, 16)
            for i in range(4):
                with tc.tile_critical():
                    nc.sync.wait_ge(csem, 16)
                    col = nc.sync.value_load(
                        capidx_sb[0:1, 2 * i + 1:2 * i + 2])
                    nc.sync.dma_start(
                        capt[:, i:i + 1],
                        astore_dram[:, bass.ds(col, 1)],
                    ).then_inc(csem, 16)

            # lane select: mask then ones-reduce over partitions
            with tc.tile_critical():
                nc.vector.wait_ge(csem, 80)
                nc.vector.tensor_tensor(
                    out=capt[:], in0=capt[:], in1=lanemask_sb[:], op=AL.mult)
            nc.tensor.matmul(
                psum_cap[:], lhsT=ones_sb[0:NLANE, 0:1], rhs=capt[:],
                start=True, stop=True)
            nc.scalar.copy(smalls[0:1, 0:4], psum_cap[:])

            # ---- loss assembly ----
            nc.vector.tensor_tensor(
                out=smalls[0:1, 4:6], in0=smalls[0:1, 0:4:2],
                in1=smalls[0:1, 1:4:2], op=AL.add)
            nc.scalar.activation(smalls[0:1, 6:8], smalls[0:1, 4:6], AF.Ln)
            nc.vector.tensor_tensor(
                out=smalls[0:1, 8:10], in0=smalls[0:1, 6:8],
                in1=constc_sb[0:1, 0:2], op=AL.add)
            nc.vector.tensor_tensor(
                out=smalls[0:1, 10:12], in0=smalls[0:1, 8:10],
                in1=smalls[0:1, 14:16], op=AL.subtract)
            nc.vector.tensor_scalar_mul(
                smalls[0:1, 12:14], smalls[0:1, 10:12], -1.0)
            nc.sync.dma_start(loss_d[:], smalls[0:1, 12:14])

    nc.compile()
    es.close()
    _PROGRAM = nc
    return nc


def _host_aux(core, targets, input_lengths, target_lengths):
    """Per-core aux input arrays (index/length metadata only)."""
    b0, b1 = 2 * core, 2 * core + 1
    out = {}
    out["targ"] = np.concatenate(
        [targets[b0], targets[b1]]).astype(np.int32).reshape(1, 2 * L)

    wfac = np.zeros((NLANE, L * TC), np.float32)
    wfacpb = np.zeros((NLANE, TC), np.float32)
    skipg = np.ones((NLANE, NSKEW), np.float32)
    masklse = np.zeros((128, 14), np.float32)
    constc = np.zeros((1, 2), np.float32)
    capidx = np.zeros((1, 8), np.int32)
    lanemask = np.zeros((NLANE, 4), np.float32)

    for bl, b in ((0, b0), (1, b1)):
        tl_b = int(target_lengths[b])
        il_b = int(input_lengths[b])
        C = _sched(tl_b, il_b)
        w = np.exp(-C).astype(np.float32)
        for j in range(NCH):
            lane = 16 * bl + j
            wt = w[j * TC:(j + 1) * TC]
            wfacpb[lane, :] = wt
            for l in range(L):
                if l < tl_b:
                    wfac[lane, l * TC:(l + 1) * TC] = wt
        rep = np.zeros(L, bool)
        rep[1:] = targets[b, 1:] == targets[b, :-1]
        for l in range(L):
            if rep[l]:
                for j in range(NCH):
                    skipg[16 * bl + j, l + j] = 0.0
        m = (np.arange(T) < il_b).astype(np.float32)
        mt = np.zeros(7 * 128, np.float32)
        mt[:T] = m
        masklse[:, bl * 7:(bl + 1) * 7] = mt.reshape(7, 128).T
        constc[0, bl] = np.float32(np.sum(C[:il_b]))
        jb = (il_b - 1) // TC
        tcs = (il_b - 1) % TC
        s_last = 2 * tl_b
        w_even = s_last + 2 * jb
        w_odd = w_even - 1
        lane = 16 * bl + jb
        capidx[0, 4 * bl + 0] = lane
        capidx[0, 4 * bl + 1] = (w_even + 2) * LAYW + 1 + tcs
        capidx[0, 4 * bl + 2] = lane
        capidx[0, 4 * bl + 3] = (w_odd + 2) * LAYW + 1 + tcs
        lanemask[lane, 2 * bl + 0] = 1.0
        lanemask[lane, 2 * bl + 1] = 1.0

    out["wfac"] = wfac
    out["wfacpb"] = wfacpb
    out["skipg"] = skipg
    out["masklse"] = masklse
    out["constc"] = constc
    out["capidx"] = capidx
    out["lanemask"] = lanemask
    seedz = np.zeros((NLANE, 2), np.float32)
    seedz[:, 1] = 1.0
    seedz[0, 0] = 1.0
    seedz[16, 0] = 1.0
    seedz[0, 1] = 0.0
    seedz[16, 1] = 0.0
    out["seedz"] = seedz
    return out


def kernel(logits, targets, input_lengths, target_lengths):
    logits = np.ascontiguousarray(np.asarray(logits, dtype=np.float32))
    targets = np.asarray(targets)
    input_lengths = np.asarray(input_lengths)
    target_lengths = np.asarray(target_lengths)

    from concourse.bass_utils import run_bass_kernel_spmd

    nc = _build_program()
    in_maps = []
    for core in range(8):
        m = _host_aux(core, targets, input_lengths, target_lengths)
        m["logits"] = logits[2 * core:2 * core + 2]
        in_maps.append(m)

    res = run_bass_kernel_spmd(nc, in_maps, core_ids=list(range(8)))
    outs = res.results
    loss = np.concatenate([outs[c]["loss"].reshape(2) for c in range(8)])
    return loss.astype(np.float32)
